# revision 11
# baseline (speedup 1.0000x reference)
"""Trainium2 Bass kernel for nn_AnnCloseModel (hydrology ANN closure model).

Reference per timestep t (serial scan over nt=365, carry yt (ngrid,1)):
    z_t  = where(isnan(y_obs_t), yhat_{t-1}, y_obs_t)     # fillObs
    h    = relu([x_t, z_t] @ Wi.T + bi)
    yhat_t = (h @ Wh.T + bh) @ Wo.T + bo

Algebraic folds (host-side):
  * No nonlinearity between Wh and Wo:  yhat = relu(.) @ Wc.T + bc,
    Wc = Wo@Wh (256,), bc = Wo@bh+bo (scalar).
  * z_t = y_clean_t + mask_t * yhat_{t-1}  (y_clean = nan_to_num(y), mask = isnan(y))
  * u = Wi16.T x + bi + wiy*(y_clean + mask*(py_prev + bc)) where py = yhat - bc
    -> K=18 matmul on pure inputs [y_clean; x(16); mask] with stationary rows
       [wiy; Wi16.T; bc*wiy], plus ONE accumulating K=1 matmul wiy (x) (mask.py_prev)
       whose moving operand is lane-aligned with the col-tiled mm3 output.
  * bi folded into the relu op (per-partition bias), bc added on host to outputs.

Device layout (per core; grid shard padded to 3840 = 2 halves x 4 groups x 480):
  hidden-on-partitions; grid on the free axis. Grid chunk (h, cg) occupies
  partition group 32*cg (rows +0 y_clean, +1..16 x, +17 mask) at free h*480.
  mm1/fb row-tiled at tile_position (32*cg, 0) (K<=32 -> 4 concurrent groups);
  mm3 (Wc dot) col-tiled at (0, 32*cg) so chunk cg's output lands on lane 32*cg.
Sharding: pure data parallelism over ngrid across 8 cores; no collectives.
"""

import os
import numpy as np

NT, NGRID, NX = 365, 30000, 16
HIDDEN = 256
NCORES = 8
GSH = 3840                     # padded grid rows per core
CH = 480                       # matmul free-dim chunk
NGRP = 4                       # partition groups (tile positions 32*cg)
NHALF = 2

_RELU_ACT = int(os.environ.get("RELU_ACT", "4"))   # of 8 relu ops, how many on ScalarE


def _legalize_sync(nc, max_waits=1):
    """This walrus build rejects instructions carrying more than one sync wait
    ("Too many sync wait commands"); hoist excess waits onto preceding NoOps."""
    import concourse.mybir as mybir

    n_new = 0
    for fn in nc.m.functions:
        for bb in fn.blocks:
            out = []
            changed = False
            for inst in bb.instructions:
                si = inst.sync_info
                if si is not None and si.on_wait and len(si.on_wait) > max_waits:
                    waits = list(si.on_wait)
                    head, tail = waits[:-max_waits], waits[-max_waits:]
                    for i, w in enumerate(head):
                        out.append(
                            mybir.InstNoOp(
                                name=f"{inst.name}-syncfix{i}",
                                sync_info=mybir.SyncInfo(on_wait=[w], on_update=[]),
                                bass_nofuse=True,
                                engine=inst.engine,
                            )
                        )
                        n_new += 1
                    inst.sync_info = mybir.SyncInfo(
                        on_wait=tail, on_update=list(si.on_update or [])
                    )
                    changed = True
                out.append(inst)
            if changed:
                bb.instructions = out
    return n_new


def _build_nc(nt, bc):
    from contextlib import ExitStack

    import concourse.bass as bass
    import concourse.mybir as mybir
    import concourse.tile as tile

    F32 = mybir.dt.float32
    BF16 = mybir.dt.bfloat16
    AF = mybir.ActivationFunctionType
    ALU = mybir.AluOpType

    nc = bass.Bass(trn_type="TRN2")
    xym = nc.dram_tensor("xym", (nt, 128, 2 * CH), BF16, kind="ExternalInput")
    msk = nc.dram_tensor("msk", (nt, NGRP, 1, 2 * CH), BF16, kind="ExternalInput")
    w1d = nc.dram_tensor("w1", (128, 256), BF16, kind="ExternalInput")
    wcd = nc.dram_tensor("wc", (128, 64), BF16, kind="ExternalInput")
    outd = nc.dram_tensor("out", (nt, NGRP, NHALF, CH), F32, kind="ExternalOutput")

    ALU = mybir.AluOpType

    with tile.TileContext(nc) as tc, ExitStack() as ctx:
        const = ctx.enter_context(tc.tile_pool(name="const", bufs=1))
        rhs_pool = ctx.enter_context(tc.tile_pool(name="rhs", bufs=4))
        ps_pool = ctx.enter_context(tc.tile_pool(name="ps", bufs=3, space="PSUM"))
        py_pool = ctx.enter_context(tc.tile_pool(name="py", bufs=2, space="PSUM"))
        r_pool = ctx.enter_context(tc.tile_pool(name="r", bufs=6))
        ym_pool = ctx.enter_context(tc.tile_pool(name="ym", bufs=4))
        yh_pool = ctx.enter_context(tc.tile_pool(name="yh", bufs=4))

        w1 = const.tile([128, 256], BF16)
        nc.sync.dma_start(w1[:, :], w1d[:, :])
        wc = const.tile([128, 64], BF16)
        nc.sync.dma_start(wc[:, :], wcd[:, :])
        # persistent mask tile, double-buffered by t parity; memset once so the
        # unused partition rows are 0.0 (they feed a full-width multiply).
        mka = const.tile([128, 2 * 2 * CH], BF16)
        nc.vector.memset(mka[:, :], 0.0)

        rhs_tiles = {}

        def load_step(t):
            rhs = rhs_pool.tile([128, 2 * CH], BF16, tag="rhs")
            mk = mka[:, (t % 2) * 2 * CH : (t % 2 + 1) * 2 * CH]
            # one contiguous 128-partition DMA (host pads each group to 32
            # rows) — four separate 19-partition transfers serialize on the
            # SDMA rings at a fraction of the port bandwidth, and a grouped
            # partition-strided view breaks the scheduler's write tracking
            nc.sync.dma_start(rhs[:, :], xym[t])
            mk_grp = mk.rearrange("(g s) n -> g s n", s=32)[:, 0:1, :]
            nc.sync.dma_start(mk_grp, msk[t])
            rhs_tiles[t] = rhs

        load_step(0)
        yh_prev = {}   # half -> SBUF tile with yhat rows (valid on lanes 32*cg)
        py_tiles = {}
        for t in range(nt):
            rhs = rhs_tiles.pop(t)
            mk = mka[:, (t % 2) * 2 * CH : (t % 2 + 1) * 2 * CH]
            if t + 1 < nt:
                load_step(t + 1)

            for h in range(NHALF):
                # feedback: z-row update rhs[32cg+0] += mask_t * yhat_{t-1},
                # done on GPSIMD (otherwise idle) so no extra PE matmuls and
                # no DVE/ACT time — those PSUM-evacuation ports are the
                # bottleneck. mka is 0 off the 32*cg lanes so the x rows of
                # rhs are unchanged by the full-width add.
                if t > 0:
                    yhm = ym_pool.tile([128, CH], BF16, tag="ym")
                    nc.gpsimd.tensor_mul(
                        yhm[:, :], yh_prev[h][:, :], mk[:, h * CH : (h + 1) * CH]
                    )
                    rhs_h = rhs[:, h * CH : (h + 1) * CH]
                    nc.gpsimd.tensor_add(rhs_h, rhs_h, yhm[:, :])

                rtiles = {}
                for m in range(2):
                    for p in range(2):  # cg pair {2p, 2p+1}
                        ps = ps_pool.tile([128, 1024], F32, tag="ps")
                        for q in range(2):
                            cg = 2 * p + q
                            nc.tensor.matmul(
                                ps[:, q * 512 : q * 512 + CH],
                                w1[32 * cg : 32 * cg + 19, m * 128 : (m + 1) * 128],
                                rhs[32 * cg : 32 * cg + 19, h * CH : (h + 1) * CH],
                                start=True,
                                stop=True,
                                tile_position=(32 * cg, 0),
                                skip_group_check=True,
                            )
                        # relu + bias: r = max(ps + bi, 0), PSUM -> SBUF.
                        # Split across ACT/DVE: the PSUM read ports of these
                        # two engines are the evacuation bottleneck, and the
                        # pair feeding each mm3 lands on different engines.
                        r = r_pool.tile([128, 1024], BF16, tag="r")
                        ps_v = ps[:, :].rearrange("p (b j) -> p b j", j=512)[:, :, 0:CH]
                        r_v = r[:, :].rearrange("p (b j) -> p b j", j=512)[:, :, 0:CH]
                        if m == p:
                            nc.scalar.activation(r_v, ps_v, AF.Relu)
                        else:
                            nc.vector.tensor_scalar_max(r_v, ps_v, 0.0)
                        rtiles[(m, p)] = r

                # mm3: py = Wc . r (col-tiled; chunk cg -> lane 32*cg)
                py = py_pool.tile([128, CH], F32, tag="py")
                for p in range(2):
                    for q in range(2):
                        cg = 2 * p + q
                        for m in range(2):
                            nc.tensor.matmul(
                                py[32 * cg : 32 * cg + 32, :],
                                wc[:, m * 32 : (m + 1) * 32],
                                rtiles[(m, p)][:, q * 512 : q * 512 + CH],
                                start=(m == 0),
                                stop=(m == 1),
                                tile_position=(0, 32 * cg),
                                skip_group_check=True,
                            )
                py_tiles[h] = py

            # store yhat = py + bc (PSUM -> SBUF, then DMA). Deferred to the
            # end of the step and split across engines so neither blocks the
            # other half's relus (engine queues are strict FIFO).
            for h in range(NHALF):
                yh = yh_pool.tile([128, CH], F32, tag="yh")
                if h == 0:
                    nc.scalar.activation(yh[:, :], py_tiles[h][:, :], AF.Copy,
                                         bias=float(bc))
                else:
                    nc.vector.tensor_scalar_add(yh[:, :], py_tiles[h][:, :],
                                                float(bc))
                yhv = yh[:, :].rearrange("(g s) n -> g s n", s=32)[:, 0, :]
                nc.sync.dma_start(outd[t, :, h, :], yhv)
                yh_prev[h] = yh

    return nc


def _prep_core_inputs(x_c, y_c):
    """x_c (nt, GSH, 16) f32, y_c (nt, GSH) f32 (NaN = missing) -> xym, msk (bf16)."""
    import ml_dtypes

    nt = x_c.shape[0]
    # 32 rows per group (19 used + 13 zero pad) so the per-step load is one
    # contiguous 128-partition DMA
    xym = np.zeros((nt, NGRP, 32, 2 * CH), dtype=np.float32)
    # grid index G = (h*NGRP + cg)*CH + j  <->  xym[t, cg, :, h*CH + j]
    xv = x_c.reshape(nt, NHALF, NGRP, CH, NX)  # [t, h, cg, j, f]
    xym[:, :, 1:17, :] = xv.transpose(0, 2, 4, 1, 3).reshape(nt, NGRP, NX, 2 * CH)
    yv = y_c.reshape(nt, NHALF, NGRP, CH)
    y_clean = np.nan_to_num(yv, nan=0.0, posinf=None, neginf=None)
    mask = np.isnan(yv).astype(np.float32)
    xym[:, :, 0, :] = y_clean.transpose(0, 2, 1, 3).reshape(nt, NGRP, 2 * CH)
    mk = mask.transpose(0, 2, 1, 3).reshape(nt, NGRP, 1, 2 * CH)
    xym[:, :, 18, :] = 1.0
    return (
        np.ascontiguousarray(xym.reshape(nt, 128, 2 * CH)).astype(
            ml_dtypes.bfloat16
        ),
        np.ascontiguousarray(mk).astype(ml_dtypes.bfloat16),
    )


def _prep_weights(Wi, bi, Wh, bh, Wo, bo):
    Wc = (Wo.astype(np.float64) @ Wh.astype(np.float64)).reshape(HIDDEN)
    bc = float(
        (Wo.astype(np.float64) @ bh.astype(np.float64) + bo.astype(np.float64))[0]
    )
    wiy = Wi[:, NX].astype(np.float64)
    # stationary rows per group: {0: wiy, 1..16: Wi16.T, 17: bc*wiy, 18: bi}
    W1full = np.empty((19, HIDDEN), dtype=np.float32)
    W1full[0] = wiy
    W1full[1:17] = Wi[:, :NX].T
    # row 17 (bc*wiy*mask) is dead: the feedback operand is mask*yhat with
    # yhat = py + bc, so the bc term rides the K=1 feedback matmul instead.
    W1full[17] = 0.0
    W1full[18] = bi
    import ml_dtypes

    w1 = np.zeros((128, 256), dtype=ml_dtypes.bfloat16)
    for cg in range(NGRP):
        w1[32 * cg : 32 * cg + 19] = W1full.astype(ml_dtypes.bfloat16)
    # Wc replicated to 32 stationary columns per block so mm3 writes all
    # 128 psum partitions (avoids uninitialized lanes; same cycle cost)
    wcm = np.zeros((128, 64), dtype=ml_dtypes.bfloat16)
    wcm[:, 0:32] = Wc[:128, None].astype(ml_dtypes.bfloat16)
    wcm[:, 32:64] = Wc[128:, None].astype(ml_dtypes.bfloat16)
    bib = None  # bi folded into the ones-row of the stationary
    return w1, wcm, bib, bc


def build_for_timing(x, y, Wi, bi, Wh, bh, Wo, bo):
    """Return (nc, in_maps) for the timed runner (mytime.py)."""
    x = np.asarray(x, dtype=np.float32)
    y = np.asarray(y, dtype=np.float32)
    nt = x.shape[0]
    ngrid = x.shape[1]
    w1, wcm, bib, bc = _prep_weights(
        np.asarray(Wi, np.float32),
        np.asarray(bi, np.float32),
        np.asarray(Wh, np.float32),
        np.asarray(bh, np.float32),
        np.asarray(Wo, np.float32),
        np.asarray(bo, np.float32),
    )
    gpc = ngrid // NCORES
    in_maps = []
    for c in range(NCORES):
        x_c = np.zeros((nt, GSH, NX), dtype=np.float32)
        y_c = np.zeros((nt, GSH), dtype=np.float32)
        x_c[:, :gpc] = x[:, c * gpc : (c + 1) * gpc, :]
        y_c[:, :gpc] = y[:, c * gpc : (c + 1) * gpc, 0]
        xym, mk = _prep_core_inputs(x_c, y_c)
        in_maps.append({"xym": xym, "msk": mk, "w1": w1, "wc": wcm})
    nc = _build_nc(nt, bc)
    _legalize_sync(nc)
    return nc, in_maps


def kernel(x, y, Wi, bi, Wh, bh, Wo, bo):
    from concourse.bass_utils import run_bass_kernel_spmd

    x = np.asarray(x, dtype=np.float32)
    y = np.asarray(y, dtype=np.float32)
    nt = x.shape[0]
    ngrid = x.shape[1]

    w1, wcm, bib, bc = _prep_weights(
        np.asarray(Wi, np.float32),
        np.asarray(bi, np.float32),
        np.asarray(Wh, np.float32),
        np.asarray(bh, np.float32),
        np.asarray(Wo, np.float32),
        np.asarray(bo, np.float32),
    )

    gpc = ngrid // NCORES
    in_maps = []
    for c in range(NCORES):
        x_c = np.zeros((nt, GSH, NX), dtype=np.float32)
        y_c = np.zeros((nt, GSH), dtype=np.float32)
        x_c[:, :gpc] = x[:, c * gpc : (c + 1) * gpc, :]
        y_c[:, :gpc] = y[:, c * gpc : (c + 1) * gpc, 0]
        xym, mk = _prep_core_inputs(x_c, y_c)
        in_maps.append({"xym": xym, "msk": mk, "w1": w1, "wc": wcm})

    nc = _build_nc(nt, bc)
    _legalize_sync(nc)
    results = run_bass_kernel_spmd(nc, in_maps, core_ids=list(range(NCORES)))
    global _LAST_EXEC_NS, _LAST_RESULTS
    _LAST_EXEC_NS = results.exec_time_ns
    _LAST_RESULTS = results

    out = np.empty((nt, ngrid, 1), dtype=np.float32)
    for c in range(NCORES):
        # (nt, NGRP, NHALF, CH) -> (nt, GSH):  G = (h*NGRP+cg)*CH + j
        o = results.results[c]["out"].transpose(0, 2, 1, 3).reshape(nt, GSH)
        out[:, c * gpc : (c + 1) * gpc, 0] = o[:, :gpc]
    return out



# revision 14
# speedup vs baseline: 1.5932x; 1.5932x over previous
"""Trainium2 Bass kernel for nn_AnnCloseModel (hydrology ANN closure model).

Reference per timestep t (serial scan over nt=365, carry yt (ngrid,1)):
    z_t  = where(isnan(y_obs_t), yhat_{t-1}, y_obs_t)     # fillObs
    h    = relu([x_t, z_t] @ Wi.T + bi)
    yhat_t = (h @ Wh.T + bh) @ Wo.T + bo

Algebraic folds (host-side):
  * No nonlinearity between Wh and Wo:  yhat = relu(.) @ Wc.T + bc,
    Wc = Wo@Wh (256,), bc = Wo@bh+bo (scalar).
  * z_t = y_clean_t + mask_t * yhat_{t-1}  (y_clean = nan_to_num(y), mask = isnan(y))
  * u = Wi16.T x + bi + wiy*(y_clean + mask*yhat_prev)
    -> K=19 matmul on inputs [y_clean; x(16); 0; ones] with stationary rows
       [wiy; Wi16.T; 0; bi], plus ONE accumulating K=1 matmul wiy (x)
       (mask*yhat_prev); the feedback operand comes from the SBUF copy of
       yhat (GPSIMD multiply), lane-aligned with the col-tiled mm3 output.
  * relu evacuation (PSUM->SBUF) split across ScalarE/VectorE — their two
    PSUM read ports are the structural bottleneck of the whole kernel.

Device layout (per core; grid shard padded to 3840 = 2 halves x 4 groups x 480):
  hidden-on-partitions; grid on the free axis. Grid chunk (h, cg) occupies
  partition group 32*cg (rows +0 y_clean, +1..16 x, +17 mask) at free h*480.
  mm1/fb row-tiled at tile_position (32*cg, 0) (K<=32 -> 4 concurrent groups);
  mm3 (Wc dot) col-tiled at (0, 32*cg) so chunk cg's output lands on lane 32*cg.
Sharding: pure data parallelism over ngrid across 8 cores; no collectives.
"""

import numpy as np

NT, NGRID, NX = 365, 30000, 16
HIDDEN = 256
NCORES = 8
GSH = 3840                     # padded grid rows per core
CH = 480                       # matmul free-dim chunk
NGRP = 4                       # partition groups (tile positions 32*cg)
NHALF = 2


def _legalize_sync(nc, max_waits=1):
    """This walrus build rejects instructions carrying more than one sync wait
    ("Too many sync wait commands"); hoist excess waits onto preceding NoOps."""
    import concourse.mybir as mybir

    n_new = 0
    for fn in nc.m.functions:
        for bb in fn.blocks:
            out = []
            changed = False
            for inst in bb.instructions:
                si = inst.sync_info
                if si is not None and si.on_wait and len(si.on_wait) > max_waits:
                    waits = list(si.on_wait)
                    head, tail = waits[:-max_waits], waits[-max_waits:]
                    for i, w in enumerate(head):
                        out.append(
                            mybir.InstNoOp(
                                name=f"{inst.name}-syncfix{i}",
                                sync_info=mybir.SyncInfo(on_wait=[w], on_update=[]),
                                bass_nofuse=True,
                                engine=inst.engine,
                            )
                        )
                        n_new += 1
                    inst.sync_info = mybir.SyncInfo(
                        on_wait=tail, on_update=list(si.on_update or [])
                    )
                    changed = True
                out.append(inst)
            if changed:
                bb.instructions = out
    return n_new


def _build_nc(nt, bc):
    from contextlib import ExitStack

    import concourse.bass as bass
    import concourse.mybir as mybir
    import concourse.tile as tile

    F32 = mybir.dt.float32
    BF16 = mybir.dt.bfloat16
    AF = mybir.ActivationFunctionType
    ALU = mybir.AluOpType

    nc = bass.Bass(trn_type="TRN2")
    xym = nc.dram_tensor("xym", (nt, 128, 2 * CH), BF16, kind="ExternalInput")
    msk = nc.dram_tensor("msk", (nt, NGRP, 1, 2 * CH), BF16, kind="ExternalInput")
    w1d = nc.dram_tensor("w1", (128, 256), BF16, kind="ExternalInput")
    wcd = nc.dram_tensor("wc", (128, 64), BF16, kind="ExternalInput")
    outd = nc.dram_tensor("out", (nt, NGRP, NHALF, CH), F32, kind="ExternalOutput")

    ALU = mybir.AluOpType

    with tile.TileContext(nc) as tc, ExitStack() as ctx:
        const = ctx.enter_context(tc.tile_pool(name="const", bufs=1))
        rhs_pool = ctx.enter_context(tc.tile_pool(name="rhs", bufs=4))
        ps_pool = ctx.enter_context(tc.tile_pool(name="ps", bufs=3, space="PSUM"))
        py_pool = ctx.enter_context(tc.tile_pool(name="py", bufs=2, space="PSUM"))
        r_pool = ctx.enter_context(tc.tile_pool(name="r", bufs=6))
        ym_pool = ctx.enter_context(tc.tile_pool(name="ym", bufs=4))
        yh_pool = ctx.enter_context(tc.tile_pool(name="yh", bufs=4))

        w1 = const.tile([128, 256], BF16)
        nc.sync.dma_start(w1[:, :], w1d[:, :])
        wc = const.tile([128, 64], BF16)
        nc.sync.dma_start(wc[:, :], wcd[:, :])
        # persistent mask tile, double-buffered by t parity; memset once so the
        # unused partition rows are 0.0 (they feed a full-width multiply).
        mka = const.tile([128, 2 * 2 * CH], BF16)
        nc.vector.memset(mka[:, :], 0.0)

        rhs_tiles = {}

        def load_step(t):
            rhs = rhs_pool.tile([128, 2 * CH], BF16, tag="rhs")
            mk = mka[:, (t % 2) * 2 * CH : (t % 2 + 1) * 2 * CH]
            # one contiguous 128-partition DMA (host pads each group to 32
            # rows) — four separate 19-partition transfers serialize on the
            # SDMA rings at a fraction of the port bandwidth, and a grouped
            # partition-strided view breaks the scheduler's write tracking
            nc.sync.dma_start(rhs[:, :], xym[t])
            mk_grp = mk.rearrange("(g s) n -> g s n", s=32)[:, 0:1, :]
            nc.sync.dma_start(mk_grp, msk[t])
            rhs_tiles[t] = rhs

        load_step(0)
        yh_prev = {}   # half -> SBUF tile with yhat rows (valid on lanes 32*cg)
        py_tiles = {}
        for t in range(nt):
            rhs = rhs_tiles.pop(t)
            mk = mka[:, (t % 2) * 2 * CH : (t % 2 + 1) * 2 * CH]
            if t + 1 < nt:
                load_step(t + 1)

            for h in range(NHALF):
                # feedback operand: yhm = mask_t * yhat_{t-1}; yhat from the
                # SBUF copy of the previous step (GPSIMD keeps DVE/ACT free
                # for the PSUM relu evacuation, which is the bottleneck).
                # mka is 0 off the 32*cg lanes, so yhm is too.
                yhm = None
                if t > 0:
                    yhm = ym_pool.tile([128, CH], BF16, tag="ym")
                    nc.gpsimd.tensor_mul(
                        yhm[:, :], yh_prev[h][:, :], mk[:, h * CH : (h + 1) * CH]
                    )

                rtiles = {}
                for m in range(2):
                    for p in range(2):  # cg pair {2p, 2p+1}
                        ps = ps_pool.tile([128, 1024], F32, tag="ps")
                        for q in range(2):
                            cg = 2 * p + q
                            nc.tensor.matmul(
                                ps[:, q * 512 : q * 512 + CH],
                                w1[32 * cg : 32 * cg + 19, m * 128 : (m + 1) * 128],
                                rhs[32 * cg : 32 * cg + 19, h * CH : (h + 1) * CH],
                                start=True,
                                stop=(t == 0),
                                tile_position=(32 * cg, 0),
                                skip_group_check=True,
                            )
                            if t > 0:
                                nc.tensor.matmul(
                                    ps[:, q * 512 : q * 512 + CH],
                                    w1[32 * cg : 32 * cg + 1, m * 128 : (m + 1) * 128],
                                    yhm[32 * cg : 32 * cg + 1, :],
                                    start=False,
                                    stop=True,
                                    tile_position=(32 * cg, 0),
                                    skip_group_check=True,
                                )
                        # relu + bias: r = max(ps + bi, 0), PSUM -> SBUF.
                        # Split across ACT/DVE: the PSUM read ports of these
                        # two engines are the evacuation bottleneck, and the
                        # pair feeding each mm3 lands on different engines.
                        r = r_pool.tile([128, 1024], BF16, tag="r")
                        ps_v = ps[:, :].rearrange("p (b j) -> p b j", j=512)[:, :, 0:CH]
                        r_v = r[:, :].rearrange("p (b j) -> p b j", j=512)[:, :, 0:CH]
                        if m == p:
                            nc.scalar.activation(r_v, ps_v, AF.Relu)
                        else:
                            nc.vector.tensor_scalar_max(r_v, ps_v, 0.0)
                        rtiles[(m, p)] = r

                # mm3: py = Wc . r (col-tiled; chunk cg -> lane 32*cg)
                py = py_pool.tile([128, CH], F32, tag="py")
                for p in range(2):
                    for q in range(2):
                        cg = 2 * p + q
                        for m in range(2):
                            nc.tensor.matmul(
                                py[32 * cg : 32 * cg + 32, :],
                                wc[:, m * 32 : (m + 1) * 32],
                                rtiles[(m, p)][:, q * 512 : q * 512 + CH],
                                start=(m == 0),
                                stop=(m == 1),
                                tile_position=(0, 32 * cg),
                                skip_group_check=True,
                            )
                py_tiles[h] = py

            # store yhat = py + bc (PSUM -> SBUF, then DMA). Deferred to the
            # end of the step and split across engines so neither blocks the
            # other half's relus (engine queues are strict FIFO).
            for h in range(NHALF):
                yh = yh_pool.tile([128, CH], F32, tag="yh")
                if h == 0:
                    nc.scalar.activation(yh[:, :], py_tiles[h][:, :], AF.Copy,
                                         bias=float(bc))
                else:
                    nc.vector.tensor_scalar_add(yh[:, :], py_tiles[h][:, :],
                                                float(bc))
                yhv = yh[:, :].rearrange("(g s) n -> g s n", s=32)[:, 0, :]
                nc.sync.dma_start(outd[t, :, h, :], yhv)
                yh_prev[h] = yh

    return nc


def _prep_core_inputs(x_c, y_c):
    """x_c (nt, GSH, 16) f32, y_c (nt, GSH) f32 (NaN = missing) -> xym, msk (bf16)."""
    import ml_dtypes

    nt = x_c.shape[0]
    # 32 rows per group (19 used + 13 zero pad) so the per-step load is one
    # contiguous 128-partition DMA
    xym = np.zeros((nt, NGRP, 32, 2 * CH), dtype=np.float32)
    # grid index G = (h*NGRP + cg)*CH + j  <->  xym[t, cg, :, h*CH + j]
    xv = x_c.reshape(nt, NHALF, NGRP, CH, NX)  # [t, h, cg, j, f]
    xym[:, :, 1:17, :] = xv.transpose(0, 2, 4, 1, 3).reshape(nt, NGRP, NX, 2 * CH)
    yv = y_c.reshape(nt, NHALF, NGRP, CH)
    y_clean = np.nan_to_num(yv, nan=0.0, posinf=None, neginf=None)
    mask = np.isnan(yv).astype(np.float32)
    xym[:, :, 0, :] = y_clean.transpose(0, 2, 1, 3).reshape(nt, NGRP, 2 * CH)
    mk = mask.transpose(0, 2, 1, 3).reshape(nt, NGRP, 1, 2 * CH)
    xym[:, :, 18, :] = 1.0
    return (
        np.ascontiguousarray(xym.reshape(nt, 128, 2 * CH)).astype(
            ml_dtypes.bfloat16
        ),
        np.ascontiguousarray(mk).astype(ml_dtypes.bfloat16),
    )


def _prep_weights(Wi, bi, Wh, bh, Wo, bo):
    Wc = (Wo.astype(np.float64) @ Wh.astype(np.float64)).reshape(HIDDEN)
    bc = float(
        (Wo.astype(np.float64) @ bh.astype(np.float64) + bo.astype(np.float64))[0]
    )
    wiy = Wi[:, NX].astype(np.float64)
    # stationary rows per group: {0: wiy, 1..16: Wi16.T, 17: bc*wiy, 18: bi}
    W1full = np.empty((19, HIDDEN), dtype=np.float32)
    W1full[0] = wiy
    W1full[1:17] = Wi[:, :NX].T
    # row 17 (bc*wiy*mask) is dead: the feedback operand is mask*yhat with
    # yhat = py + bc, so the bc term rides the K=1 feedback matmul instead.
    W1full[17] = 0.0
    W1full[18] = bi
    import ml_dtypes

    w1 = np.zeros((128, 256), dtype=ml_dtypes.bfloat16)
    for cg in range(NGRP):
        w1[32 * cg : 32 * cg + 19] = W1full.astype(ml_dtypes.bfloat16)
    # Wc replicated to 32 stationary columns per block so mm3 writes all
    # 128 psum partitions (avoids uninitialized lanes; same cycle cost)
    wcm = np.zeros((128, 64), dtype=ml_dtypes.bfloat16)
    wcm[:, 0:32] = Wc[:128, None].astype(ml_dtypes.bfloat16)
    wcm[:, 32:64] = Wc[128:, None].astype(ml_dtypes.bfloat16)
    bib = None  # bi folded into the ones-row of the stationary
    return w1, wcm, bib, bc


def build_for_timing(x, y, Wi, bi, Wh, bh, Wo, bo):
    """Return (nc, in_maps) for the timed runner (mytime.py)."""
    x = np.asarray(x, dtype=np.float32)
    y = np.asarray(y, dtype=np.float32)
    nt = x.shape[0]
    ngrid = x.shape[1]
    w1, wcm, bib, bc = _prep_weights(
        np.asarray(Wi, np.float32),
        np.asarray(bi, np.float32),
        np.asarray(Wh, np.float32),
        np.asarray(bh, np.float32),
        np.asarray(Wo, np.float32),
        np.asarray(bo, np.float32),
    )
    gpc = ngrid // NCORES
    in_maps = []
    for c in range(NCORES):
        x_c = np.zeros((nt, GSH, NX), dtype=np.float32)
        y_c = np.zeros((nt, GSH), dtype=np.float32)
        x_c[:, :gpc] = x[:, c * gpc : (c + 1) * gpc, :]
        y_c[:, :gpc] = y[:, c * gpc : (c + 1) * gpc, 0]
        xym, mk = _prep_core_inputs(x_c, y_c)
        in_maps.append({"xym": xym, "msk": mk, "w1": w1, "wc": wcm})
    nc = _build_nc(nt, bc)
    _legalize_sync(nc)
    return nc, in_maps


def kernel(x, y, Wi, bi, Wh, bh, Wo, bo):
    from concourse.bass_utils import run_bass_kernel_spmd

    x = np.asarray(x, dtype=np.float32)
    y = np.asarray(y, dtype=np.float32)
    nt = x.shape[0]
    ngrid = x.shape[1]

    w1, wcm, bib, bc = _prep_weights(
        np.asarray(Wi, np.float32),
        np.asarray(bi, np.float32),
        np.asarray(Wh, np.float32),
        np.asarray(bh, np.float32),
        np.asarray(Wo, np.float32),
        np.asarray(bo, np.float32),
    )

    gpc = ngrid // NCORES
    in_maps = []
    for c in range(NCORES):
        x_c = np.zeros((nt, GSH, NX), dtype=np.float32)
        y_c = np.zeros((nt, GSH), dtype=np.float32)
        x_c[:, :gpc] = x[:, c * gpc : (c + 1) * gpc, :]
        y_c[:, :gpc] = y[:, c * gpc : (c + 1) * gpc, 0]
        xym, mk = _prep_core_inputs(x_c, y_c)
        in_maps.append({"xym": xym, "msk": mk, "w1": w1, "wc": wcm})

    nc = _build_nc(nt, bc)
    _legalize_sync(nc)
    results = run_bass_kernel_spmd(nc, in_maps, core_ids=list(range(NCORES)))
    global _LAST_EXEC_NS, _LAST_RESULTS
    _LAST_EXEC_NS = results.exec_time_ns
    _LAST_RESULTS = results

    out = np.empty((nt, ngrid, 1), dtype=np.float32)
    for c in range(NCORES):
        # (nt, NGRP, NHALF, CH) -> (nt, GSH):  G = (h*NGRP+cg)*CH + j
        o = results.results[c]["out"].transpose(0, 2, 1, 3).reshape(nt, GSH)
        out[:, c * gpc : (c + 1) * gpc, 0] = o[:, :gpc]
    return out

